# revision 5
# baseline (speedup 1.0000x reference)
import numpy as np
import jax
import jax.numpy as jnp

# nn_Atom91Head: SE(3) GNN message-passing head on 8 NeuronCores.
# Sharding: edges split E/8 per core; node features + weights replicated
# (node tensors are tiny, so replication beats halo exchange). Per-core
# segment_sum over full N, AllReduce (psum) across cores, then the cheap
# post/refine stage runs replicated; core 0's copy is returned.
# The graph is split into 3 pmap stages — the fully-fused single graph
# tickles a neuronx-cc INTERNAL compile error; the staged graphs compile.

N, E = 4096, 32768
D0, D1, AA, COUT = 32, 16, 20, 91
EDGE_DIM, MID = 17, 32
NORM_CLAMP = 2.0 ** -24
EPS = 1e-5
NCORES = 8

_NODE_KEYS = ("density_0", "density_1", "seq_0")
_EDGE_KEYS = ("edge_scalars", "basis_01", "basis_11", "src", "dst")
_W_KEYS = ("Wt0", "Wt1", "ln0_w", "ln0_b", "ln1_w", "ln1_b",
           "r01_W1", "r01_b1", "r01_W2", "r01_b2", "r01_W3",
           "r11_W1", "r11_b1", "r11_W2", "r11_b2", "r11_W3",
           "W_self", "gn_w", "gn_b", "ref_Wa", "ref_gn_w", "ref_gn_b", "ref_Wb")


def _norm_se3(feat, w, b):
    norm = jnp.clip(jnp.linalg.norm(feat, axis=-1, keepdims=True), NORM_CLAMP)
    n = norm[..., 0]
    mu = jnp.mean(n, axis=-1, keepdims=True)
    var = jnp.mean((n - mu) ** 2, axis=-1, keepdims=True)
    new_n = jax.nn.relu((n - mu) / jnp.sqrt(var + EPS) * w + b)
    return feat / norm * new_n[..., None]


def _radial2(es, W1, b1, W2, b2):
    h = jax.nn.relu(es @ W1.T + b1)
    return jax.nn.relu(h @ W2.T + b2)


def _stage_a(density_0, density_1, seq_0, edge_scalars, src,
             Wt0, Wt1, ln0_w, ln0_b, ln1_w, ln1_b,
             r01_W1, r01_b1, r01_W2, r01_b2,
             r11_W1, r11_b1, r11_W2, r11_b2):
    f0 = jnp.concatenate([density_0, seq_0], axis=1)
    t0 = _norm_se3(jnp.einsum('oi,nim->nom', Wt0, f0), ln0_w, ln0_b)
    t1 = _norm_se3(jnp.einsum('oi,nim->nom', Wt1, density_1), ln1_w, ln1_b)
    h01 = _radial2(edge_scalars, r01_W1, r01_b1, r01_W2, r01_b2)
    h11 = _radial2(edge_scalars, r11_W1, r11_b1, r11_W2, r11_b2)
    return t0[src, :, 0], t1[src], h01, h11, t1


def _stage_b(t0s, t1s, h01, h11, basis_01, basis_11, r01_W3, r11_W3):
    z01 = (t0s[:, :, None] * h01[:, None, :]).reshape(-1, D0 * MID)
    W3r01 = r01_W3.reshape(COUT, D0, MID).transpose(1, 2, 0).reshape(D0 * MID, COUT)
    s01 = z01 @ W3r01
    edge_out = s01[:, :, None] * basis_01[:, 0, 0, :][:, None, :]
    v = jnp.einsum('eci,eifm->ecfm', t1s, basis_11)
    zv = (v.reshape(-1, D1 * 3, 1, 3) * h11[:, None, :, None]).reshape(-1, D1 * 3 * MID, 3)
    W3r11 = r11_W3.reshape(COUT, D1 * 3, MID).transpose(1, 2, 0).reshape(D1 * 3 * MID, COUT)
    return edge_out + jnp.einsum('ezm,zk->ekm', zv, W3r11)


def _stage_c(edge_out, dst, t1, W_self, gn_w, gn_b, ref_Wa, ref_gn_w, ref_gn_b, ref_Wb):
    partial = jax.ops.segment_sum(edge_out, dst, num_segments=N)
    atoms = jax.lax.psum(partial, 'd')
    atoms = atoms + jnp.einsum('oi,nim->nom', W_self, t1)
    atoms = _norm_se3(atoms, gn_w, gn_b)
    for i in range(3):
        y = jnp.einsum('oi,nim->nom', ref_Wa[i], atoms)
        y = _norm_se3(y, ref_gn_w[i], ref_gn_b[i])
        y = jnp.einsum('oi,nim->nom', ref_Wb[i], y)
        atoms = atoms + y
    return atoms


_FNS = None


def _get_fns():
    global _FNS
    if _FNS is None:
        devs = jax.devices()[:NCORES]
        pa = jax.pmap(_stage_a, in_axes=(None, None, None, 0, 0) + (None,) * 14,
                      devices=devs)
        pb = jax.pmap(_stage_b, in_axes=(0, 0, 0, 0, 0, 0, None, None), devices=devs)
        pc = jax.pmap(_stage_c, axis_name='d',
                      in_axes=(0, 0, 0) + (None,) * 7, devices=devs)
        _FNS = (pa, pb, pc)
    return _FNS


def kernel(**inputs):
    pa, pb, pc = _get_fns()
    nodes = [jnp.asarray(np.asarray(inputs[k]), jnp.float32) for k in _NODE_KEYS]
    w = {k: jnp.asarray(np.asarray(inputs[k]), jnp.float32) for k in _W_KEYS}
    eshard = {}
    for k in _EDGE_KEYS:
        a = np.asarray(inputs[k])
        if a.dtype in (np.int64, np.int32):
            a = a.astype(np.int32).reshape(NCORES, E // NCORES)
        else:
            a = a.astype(np.float32).reshape((NCORES, E // NCORES) + a.shape[1:])
        eshard[k] = jnp.asarray(a)

    t0s, t1s, h01, h11, t1 = pa(
        nodes[0], nodes[1], nodes[2], eshard["edge_scalars"], eshard["src"],
        w["Wt0"], w["Wt1"], w["ln0_w"], w["ln0_b"], w["ln1_w"], w["ln1_b"],
        w["r01_W1"], w["r01_b1"], w["r01_W2"], w["r01_b2"],
        w["r11_W1"], w["r11_b1"], w["r11_W2"], w["r11_b2"])
    edge_out = pb(t0s, t1s, h01, h11, eshard["basis_01"], eshard["basis_11"],
                  w["r01_W3"], w["r11_W3"])
    atoms = pc(edge_out, eshard["dst"], t1,
               w["W_self"], w["gn_w"], w["gn_b"],
               w["ref_Wa"], w["ref_gn_w"], w["ref_gn_b"], w["ref_Wb"])
    return np.asarray(atoms[0])
